# revision 1
# baseline (speedup 1.0000x reference)
"""Trainium2 Bass kernel for windowed 3D attention (nn_Attention_12927851561046).

512 windows of 343-token, 4-head, 32-dim-per-head attention over d=128.
Pure data parallel: 64 windows per core across 8 NeuronCores.

Layout strategy (per window):
  XT (d=128 partitions, 343 tokens free) bf16
  qT/kT = w^T@XT  -> psum -> cast to bf16 sbuf (128=4h*32dh, 343)
  v    = XT^T@wv  -> psum (t-chunks, 128) -> strided cast into vsb1
         (128, 3*132) bf16: per chunk [v_h0|1][1|v_h1][v_h2|1][1|v_h3]
  sim: 12 single-bank psum tiles (chunk x head), bufs=4.
  exp: ACT activation for heads 0-2; head 3 via fused DVE Schraudolph
       (affine_then_add -> int16 = bf16 bits of exp(sim)*eb, bias folded).
  attn = expsim * eb: h0 on DVE, h1 on Pool, h2 on DVE (readiness order).
  The whole window pipeline is software-pipelined: window w's normalize/
  projection tail is emitted interleaved with window w+1's head so the
  in-order engines never stall on it.
  ao   = [v|1]^T @ attn per (chunk, head): psum bank rows 0-32 for even
         head (v rows 0-31, rowsum row 32), rows 64-96 for odd head
         (rowsum row 64, v rows 65-96).  Softmax denominators ride the
         PV matmul -- no separate rowsum matmul pass.
  ao banks -> bf16 sbuf (ACT + DVE), rep = ind97^T @ that (K=97 matmul,
  sharing the v bank), recip on DVE, anrm = ao_bf16 * recip on Pool.
  final: per i-chunk two K=97 matmuls vs zero-padded wout97 -> psum
  (sharing the qk bank); psum -> sbuf f32 (ACT + DVE) -> DMA out.
"""

import sys
from contextlib import ExitStack

import numpy as np

sys.path.insert(0, "/opt/trn_rl_repo")

import ml_dtypes  # noqa: E402

import concourse.bass as bass  # noqa: E402
import concourse.tile as tile  # noqa: E402
from concourse.tile import add_dep_helper  # noqa: E402
from concourse import bacc, mybir  # noqa: E402
from concourse import bass_utils  # noqa: E402

BF16 = mybir.dt.bfloat16
F32 = mybir.dt.float32
I16 = mybir.dt.int16

NW = 64          # windows per core
N = 343          # tokens per window
D = 128
H = 4
DH = 32
NP = 384         # padded tokens (zeros beyond 343)
JOFF = [0, 128, 256]

# bf16-domain Schraudolph exp: bits16 = sim*SCHR_A + SCHR_B (+ leb table)
SCHR_A = 128.0 / float(np.log(2.0))   # 184.665
SCHR_B = 16248.7                      # 16256 - 7.3 (rms-calibrated)
LEB_MASK = -13000.0                   # masked rows: bits ~3250 -> ~2^-102

# column split points within a chunk's 1372 attn columns
XF = 1029        # [XF:1372] folded Schraudolph on DVE (= head 3)

# module-level knobs (test.py pokes these)
TRACE = False
TRACE_KWARGS = {}

_cache = {}


def _build_kernel():
    nc = bacc.Bacc(
        "TRN2",
        target_bir_lowering=False,
        debug=False,
        enable_asserts=False,
        num_devices=8,
    )
    xt_d = nc.dram_tensor("xt", (NW, D, NP), BF16, kind="ExternalInput").ap()
    wqkv_d = nc.dram_tensor("wqkv", (D, 3 * D), BF16, kind="ExternalInput").ap()
    wout_d = nc.dram_tensor("wout", (2 * 97, D), BF16, kind="ExternalInput").ap()
    eb_d = nc.dram_tensor("eb", (D, 3 * H * N), BF16, kind="ExternalInput").ap()
    leb_d = nc.dram_tensor("leb", (D, 3 * (4 * N - XF)), F32, kind="ExternalInput").ap()
    out_d = nc.dram_tensor("out", (NW, N, D), F32, kind="ExternalOutput").ap()

    with tile.TileContext(nc) as tc:
        with ExitStack() as ctx:
            _body(ctx, tc, out_d, xt_d, wqkv_d, wout_d, eb_d, leb_d)

    nc.compile()
    return nc


def _chain(insts):
    for a, b in zip(insts[1:], insts[:-1]):
        add_dep_helper(a.ins, b.ins, sync=False, reason="psum accumulation order")


def _ao_tile(ps, tag):
    return ps.tile([97, N], F32, tag=tag, bufs=1, padded_shape=[97, 512],
                   name=tag)


def _body(ctx, tc, out_d, xt_d, wqkv_d, wout_d, eb_d, leb_d):
    nc = tc.nc

    const = ctx.enter_context(tc.tile_pool(name="const", bufs=1))
    sb = ctx.enter_context(tc.tile_pool(name="sb", bufs=4))
    ps = ctx.enter_context(tc.tile_pool(name="ps", bufs=1, space="PSUM"))

    # constants
    wqkv = const.tile([D, 3 * D], BF16)
    nc.sync.dma_start(wqkv[:], wqkv_d[:])
    woutA = const.tile([97, D], BF16)
    nc.sync.dma_start(woutA[:], wout_d[0:97, :])
    woutB = const.tile([97, D], BF16)
    nc.sync.dma_start(woutB[:], wout_d[97:194, :])
    eb = const.tile([D, 3 * H * N], BF16)
    nc.sync.dma_start(eb[:], eb_d[:])
    leb = const.tile([D, 3 * (4 * N - XF)], F32)
    nc.sync.dma_start(leb[:], leb_d[:])
    # rowsum-replication indicator: row 32 (rs of even head) -> out rows
    # 0..63, row 64 (rs of odd head) -> out rows 64..96; all other rows zero.
    ind97 = const.tile([97, 97], BF16)
    nc.vector.memset(ind97[:], 0.0)
    nc.vector.memset(ind97[32:33, 0:64], 1.0)
    nc.vector.memset(ind97[64:65, 64:97], 1.0)

    zrow = const.tile([1, 97], BF16)
    nc.vector.memset(zrow[:], 0.0)

    # one-time zero of the ao banks (rows 33..63 are never written by the
    # per-window matmuls but are read by the rsbf cast)
    zA = _ao_tile(ps, "aoA")
    zB = _ao_tile(ps, "aoB")
    nc.tensor.matmul(zA[:, 0:N], lhsT=zrow[:], rhs=eb[0:1, 0:N],
                     start=True, stop=True)
    nc.tensor.matmul(zB[:, 0:N], lhsT=zrow[:], rhs=eb[0:1, 0:N],
                     start=True, stop=True)

    def emit_xt(w):
        xt = sb.tile([D, NP], BF16, tag="xt", name="xt")
        nc.sync.dma_start(xt[:], xt_d[w])
        return xt

    def emit_head(w, xt):
        qp = ps.tile([D, N], F32, tag="qk", bufs=1, padded_shape=[D, 512], name="qp")
        nc.tensor.matmul(qp[:], lhsT=wqkv[:, 0:D], rhs=xt[:, 0:N], start=True, stop=True)
        qsb = sb.tile([D, N], BF16, tag="qsb", name="qsb")
        nc.vector.tensor_copy(qsb[:], qp[:])          # DVE cast

        kp = ps.tile([D, NP], F32, tag="qk", bufs=1, padded_shape=[D, 512], name="kp")
        nc.tensor.matmul(kp[:], lhsT=wqkv[:, D:2 * D], rhs=xt[:], start=True, stop=True)
        ksb = sb.tile([D, NP], BF16, tag="ksb", name="ksb")
        nc.scalar.copy(ksb[:], kp[:])                 # ACT cast

        vp = ps.tile([D, 3 * D], F32, tag="qk", bufs=1, padded_shape=[D, 512], name="vp")
        v_mms = []
        for c in range(3):
            v_mms.append(nc.tensor.matmul(
                vp[:, c * D:(c + 1) * D],
                lhsT=xt[:, JOFF[c]:JOFF[c] + D],
                rhs=wqkv[:, 2 * D:3 * D],
                start=(c == 0), stop=(c == 2),
            ))
        _chain(v_mms)

        vsb1 = sb.tile([D, 3 * 132], BF16, tag="vsb1", name="vsb1")
        vdst = vsb1[:].rearrange("p (c g s) -> p c g s", c=3, g=2)   # s=66
        vsrc = vp[:].rearrange("p (c g s) -> p c g s", c=3, g=2)     # s=64
        nc.vector.tensor_copy(vdst[:, :, :, 0:32], vsrc[:, :, :, 0:32])
        nc.vector.tensor_copy(vdst[:, :, :, 34:66], vsrc[:, :, :, 32:64])
        nc.gpsimd.memset(vdst[:, :, :, 32:34], 1.0)  # ones cols on Pool
        return qsb, ksb, vsb1

    def emit_chunk_sims(qsb, ksb, c):
        expsim = sb.tile([D, XF], BF16, tag="es", bufs=6, name="es")
        attn = sb.tile([D, H * N], BF16, tag="attn", bufs=6, name="attn")
        for h in range(H):
            s = ps.tile([D, N], F32, tag="sim", bufs=4, padded_shape=[D, 512],
                        name="s")
            nc.tensor.matmul(
                s[:],
                lhsT=ksb[DH * h:DH * (h + 1), JOFF[c]:JOFF[c] + D],
                rhs=qsb[DH * h:DH * (h + 1), 0:N],
                tile_position=(DH * h, 0),
                start=True, stop=True,
            )
            if h < 3:
                nc.scalar.activation(
                    expsim[:, N * h:N * (h + 1)],
                    s[:],
                    mybir.ActivationFunctionType.Exp,
                )
            else:
                # folded Schraudolph: attn[h3] = bf16 bits of exp(sim)*eb
                nfold = 4 * N - XF
                nc.vector.affine_then_add(
                    attn[:, XF:4 * N].bitcast(I16),
                    s[:],
                    leb[:, nfold * c:nfold * (c + 1)],
                    SCHR_A, SCHR_B,
                )
        # eb multiply for the ACT-exp'd range.  Pool gets the EARLIEST
        # columns (ready after exp h0) so its slow op hides under the
        # later exps; DVE covers the rest in two ops so the last piece
        # only waits on exp h2.
        nc.vector.tensor_mul(attn[:, 0:N], expsim[:, 0:N],
                             eb[:, H * N * c:H * N * c + N])
        nc.gpsimd.tensor_mul(attn[:, N:2 * N], expsim[:, N:2 * N],
                             eb[:, H * N * c + N:H * N * c + 2 * N])
        nc.vector.tensor_mul(attn[:, 2 * N:XF], expsim[:, 2 * N:XF],
                             eb[:, H * N * c + 2 * N:H * N * c + XF])
        return attn

    def emit_chunk_aos(aoA, aoB, vsb1, attn, c, ao_mms):
        # issue order by operand readiness: h3 (DVE affine, earliest),
        # h0 (DVE mult), h1 (Pool mult), h2 (DVE mult after last exp)
        for h in (3, 0, 1, 2):
            bank = aoA if h < 2 else aoB
            off = 64 * (h % 2)
            ao_mms.append(nc.tensor.matmul(
                bank[off:off + 33, :],
                lhsT=vsb1[:, 132 * c + 33 * h:132 * c + 33 * h + 33],
                rhs=attn[:, N * h:N * (h + 1)],
                tile_position=(0, off),
                start=(c == 0), stop=(c == 2),
                skip_group_check=True,
            ))

    def emit_rsbf(aoA, aoB):
        # cast whole banks to bf16 (cost is free-size; base-0 APs only)
        rsbf = sb.tile([97, 2 * N], BF16, tag="rsbf", name="rsbf")
        nc.scalar.copy(rsbf[:, 0:N], aoA[:, 0:N])
        nc.vector.tensor_copy(rsbf[:, N:2 * N], aoB[:, 0:N])
        return rsbf

    def emit_rep(rsbf, half, nm):
        rep = ps.tile([97, N], F32, tag="fin", bufs=1, padded_shape=[97, 512],
                      name=nm)
        nc.tensor.matmul(rep[:], lhsT=ind97[:], rhs=rsbf[:, half * N:(half + 1) * N],
                         start=True, stop=True)
        rc = sb.tile([97, N], F32, tag=nm + "rc", name=nm + "rc")
        nc.vector.reciprocal_approx_fast(rc[:], rep[:])
        return rc

    def emit_anrm(rsbf, rc, half, nm):
        anrm = sb.tile([97, N], BF16, tag=nm, name=nm)
        nc.gpsimd.tensor_mul(anrm[:], rsbf[:, half * N:(half + 1) * N], rc[:])
        return anrm

    def emit_fin(w, anrmA, anrmB):
        fp = ps.tile([D, 3 * D], F32, tag="fin", bufs=1, padded_shape=[D, 512],
                     name="fp")
        f_mms = []
        for c in range(3):
            jc = min(D, N - JOFF[c])
            f_mms.append(nc.tensor.matmul(
                fp[0:jc, c * D:(c + 1) * D],
                lhsT=anrmA[:, JOFF[c]:JOFF[c] + jc],
                rhs=woutA[:],
                start=True, stop=False,
                skip_group_check=True,
            ))
            f_mms.append(nc.tensor.matmul(
                fp[0:jc, c * D:(c + 1) * D],
                lhsT=anrmB[:, JOFF[c]:JOFF[c] + jc],
                rhs=woutB[:],
                start=False, stop=True,
                skip_group_check=True,
            ))
        _chain(f_mms)

        fsb = sb.tile([D, 3 * D], F32, tag="fsb", name="fsb")
        cp1 = nc.scalar.copy(fsb[:, 0:2 * D], fp[:, 0:2 * D])
        add_dep_helper(cp1.ins, f_mms[-1].ins, sync=True,
                       reason="read after accumulation group closes")
        nc.vector.tensor_copy(fsb[0:87, 2 * D:3 * D], fp[0:87, 2 * D:3 * D])

        dst01 = out_d[w, 0:256, :].rearrange("(c p) d -> p c d", p=D)
        src01 = fsb[:, 0:256].rearrange("p (c d) -> p c d", c=2)
        nc.sync.dma_start(dst01, src01)
        nc.sync.dma_start(out_d[w, 256:343, :], fsb[0:87, 2 * D:3 * D])

    # software-pipelined main loop: window w's normalize/projection tail is
    # emitted interleaved with window w+1's head so in-order engines never
    # stall on it
    prev = None   # (w, rsbf) of the previous window
    xt_cur = emit_xt(0)
    for w in range(NW):
        # prefetch next window's input ahead of this iteration's out-DMAs
        # (the SP DMA queue is in-order)
        xt_next = emit_xt(w + 1) if w + 1 < NW else None
        qsb, ksb, vsb1 = emit_head(w, xt_cur)
        xt_cur = xt_next
        if prev is not None:
            rcA = emit_rep(prev[1], 0, "r1")
        aoA = _ao_tile(ps, "aoA")
        aoB = _ao_tile(ps, "aoB")
        ao_mms = []
        attn0 = emit_chunk_sims(qsb, ksb, 0)
        if prev is not None:
            rcB = emit_rep(prev[1], 1, "r2")
            anrmA = emit_anrm(prev[1], rcA, 0, "anrmA")
        attn1 = emit_chunk_sims(qsb, ksb, 1)
        emit_chunk_aos(aoA, aoB, vsb1, attn0, 0, ao_mms)
        if prev is not None:
            anrmB = emit_anrm(prev[1], rcB, 1, "anrmB")
        attn2 = emit_chunk_sims(qsb, ksb, 2)
        emit_chunk_aos(aoA, aoB, vsb1, attn1, 1, ao_mms)
        if prev is not None:
            emit_fin(prev[0], anrmA, anrmB)
        emit_chunk_aos(aoA, aoB, vsb1, attn2, 2, ao_mms)
        _chain(ao_mms)
        rsbf = emit_rsbf(aoA, aoB)
        prev = (w, rsbf)

    # drain the last window's tail
    rcA = emit_rep(prev[1], 0, "r1")
    rcB = emit_rep(prev[1], 1, "r2")
    anrmA = emit_anrm(prev[1], rcA, 0, "anrmA")
    anrmB = emit_anrm(prev[1], rcB, 1, "anrmB")
    emit_fin(prev[0], anrmA, anrmB)


def _prep_inputs(x, w_qkv, w_out, bias_table, rel_idx):
    x = np.asarray(x, dtype=np.float32)
    w_qkv = np.asarray(w_qkv, dtype=np.float32)
    w_out = np.asarray(w_out, dtype=np.float32)
    bias_table = np.asarray(bias_table, dtype=np.float32)
    rel_idx = np.asarray(rel_idx)

    scale = DH ** -0.5
    wq = w_qkv[:, 0:D] * scale
    wqkv_s = np.concatenate([wq, w_qkv[:, D:3 * D]], axis=1)
    wqkv_bf = wqkv_s.astype(ml_dtypes.bfloat16)

    # wout97: per bank [h_even rows | zero gap | rs row zero | h_odd rows]
    wout97 = np.zeros((194, D), dtype=np.float32)
    wout97[0:32] = w_out[0:32]          # h0 (anrm rows 0..31)
    wout97[65:97] = w_out[32:64]        # h1 (anrm rows 65..96)
    wout97[97:129] = w_out[64:96]       # h2
    wout97[162:194] = w_out[96:128]     # h3
    wout97_bf = wout97.astype(ml_dtypes.bfloat16)

    xr = x.reshape(8 * 64, N, D)
    xtf = np.zeros((8 * 64, D, NP), dtype=np.float32)
    xtf[:, :, 0:N] = xr.transpose(0, 2, 1)
    xt = xtf.astype(ml_dtypes.bfloat16).reshape(8, NW, D, NP)

    bias = bias_table[rel_idx]                     # (i, j, h)
    biasT = bias.transpose(1, 2, 0)                # (j, h, i)
    tmp = np.zeros((3 * D, H, N), dtype=np.float32)
    tmp[0:N] = np.exp(biasT)
    eb_arr = np.ascontiguousarray(
        tmp.reshape(3, D, H * N).transpose(1, 0, 2).reshape(D, 3 * H * N)
    ).astype(ml_dtypes.bfloat16)

    # leb: Schraudolph bias-fold table for attn cols [XF:4N] = head 3,
    # i in [XF-3N : N); masked (padded-j) rows get LEB_MASK
    i0 = XF - 3 * N
    ltmp = np.full((3 * D, N - i0), LEB_MASK, dtype=np.float32)
    ltmp[0:N] = SCHR_A * biasT[:, 3, i0:]
    leb_arr = np.ascontiguousarray(
        ltmp.reshape(3, D, N - i0).transpose(1, 0, 2).reshape(D, 3 * (N - i0))
    ).astype(np.float32)

    in_maps = []
    for core in range(8):
        in_maps.append({
            "xt": np.ascontiguousarray(xt[core]),
            "wqkv": wqkv_bf,
            "wout": wout97_bf,
            "eb": eb_arr,
            "leb": leb_arr,
        })
    return in_maps


def kernel(x, w_qkv, w_out, bias_table, rel_idx):
    if "nc" not in _cache:
        _cache["nc"] = _build_kernel()
    nc = _cache["nc"]
    in_maps = _prep_inputs(x, w_qkv, w_out, bias_table, rel_idx)
    res = bass_utils.run_bass_kernel_spmd(
        nc, in_maps, core_ids=list(range(8)), trace=TRACE, **TRACE_KWARGS
    )
    _cache["last_result"] = res
    outs = [res.results[c]["out"] for c in range(8)]
    full = np.concatenate(outs, axis=0)             # (512, 343, 128)
    return full.reshape(1, 8, 8, 8, 7, 7, 7, D).astype(np.float32)



# revision 2
# speedup vs baseline: 1.1349x; 1.1349x over previous
"""Trainium2 Bass kernel for windowed 3D attention (nn_Attention_12927851561046).

512 windows of 343-token, 4-head, 32-dim-per-head attention over d=128.
Pure data parallel: 64 windows per core across 8 NeuronCores.

v2 design (vs the eb-multiply baseline):
  - f16 data path everywhere bf16 was used (same matmul/DVE cost, more
    mantissa), f32 psum throughout.
  - rel-pos bias is folded into the sim PSUM *before* the QK matmul via a
    cheap fp8-e4m3 DoubleRow matmul (identity lhsT x bias-table rhs at 0.5
    cycles/row, broadcast rhs so the halved table is added twice). The
    softmax then needs NO per-element bias multiply at all.
  - heads are processed in PSUM pairs (one 2-bank tile per pair):
      pair A (h0,h1): bias-preload + QK accumulate, then ONE batched ACT
        Exp over both banks -> attn f16.
      pair B (h2,h3): QK only, then ONE batched DVE affine_then_add
        (Schraudolph: int16 bits of exp(sim+bias), bias table in f16 rides
        the in1 operand) -> attn f16.
  - PV, rowsum-rides-PV, rep/recip/anrm and the final projection keep the
    baseline structure, all in f16 (rc stays f32 for reciprocal_approx_fast).
  - Pool only does the anrm multiplies + ones memsets (GPSIMD cannot touch
    PSUM, so it cannot help with the psum->sbuf conversions).
"""

import sys
from contextlib import ExitStack

import numpy as np

sys.path.insert(0, "/opt/trn_rl_repo")

import ml_dtypes  # noqa: E402

import concourse.bass as bass  # noqa: E402
import concourse.tile as tile  # noqa: E402
from concourse.tile import add_dep_helper  # noqa: E402
from concourse import bacc, mybir  # noqa: E402
from concourse import bass_utils  # noqa: E402

F16 = mybir.dt.float16
F32 = mybir.dt.float32
I16 = mybir.dt.int16
E4 = mybir.dt.float8e4

NW = 64          # windows per core
N = 343          # tokens per window
D = 128
H = 4
DH = 32
NP = 384         # padded tokens (zeros beyond 343)
JOFF = [0, 128, 256]

# f16-domain Schraudolph exp: bits16 = sim*SCHR_A + SCHR_B (+ f16 leb table)
SCHR_A = 1024.0 / float(np.log(2.0))   # 1477.32
SCHR_B = 15360.0 - 59.0                # rms-calibrated
LEB_MASK = -60000.0                    # masked rows -> bits < 0 -> sat -> -0

TRACE = False
TRACE_KWARGS = {}

_cache = {}


def _build_kernel():
    nc = bacc.Bacc(
        "TRN2",
        target_bir_lowering=False,
        debug=False,
        enable_asserts=False,
        num_devices=8,
    )
    xt_d = nc.dram_tensor("xt", (NW, D, NP), F16, kind="ExternalInput").ap()
    wqkv_d = nc.dram_tensor("wqkv", (D, 3 * D), F16, kind="ExternalInput").ap()
    wout_d = nc.dram_tensor("wout", (2 * 97, D), F16, kind="ExternalInput").ap()
    idid_d = nc.dram_tensor("idid", (D, 2 * D), E4, kind="ExternalInput").ap()
    bias8_d = nc.dram_tensor("bias8", (D, 6 * N), E4, kind="ExternalInput").ap()
    leb_d = nc.dram_tensor("leb", (D, 6 * N), F16, kind="ExternalInput").ap()
    out_d = nc.dram_tensor("out", (NW, N, D), F32, kind="ExternalOutput").ap()

    with tile.TileContext(nc) as tc:
        with ExitStack() as ctx:
            _body(ctx, tc, out_d, xt_d, wqkv_d, wout_d, idid_d, bias8_d, leb_d)

    nc.compile()
    return nc


def _chain(insts):
    for a, b in zip(insts[1:], insts[:-1]):
        add_dep_helper(a.ins, b.ins, sync=False, reason="psum accumulation order")


def _ao_tile(ps, tag):
    return ps.tile([97, N], F32, tag=tag, bufs=1, padded_shape=[97, 512],
                   name=tag)


def _body(ctx, tc, out_d, xt_d, wqkv_d, wout_d, idid_d, bias8_d, leb_d):
    nc = tc.nc

    const = ctx.enter_context(tc.tile_pool(name="const", bufs=1))
    sb = ctx.enter_context(tc.tile_pool(name="sb", bufs=4))
    ps = ctx.enter_context(tc.tile_pool(name="ps", bufs=1, space="PSUM"))

    # constants
    wqkv = const.tile([D, 3 * D], F16)
    nc.sync.dma_start(wqkv[:], wqkv_d[:])
    woutA = const.tile([97, D], F16)
    nc.sync.dma_start(woutA[:], wout_d[0:97, :])
    woutB = const.tile([97, D], F16)
    nc.sync.dma_start(woutB[:], wout_d[97:194, :])
    idid = const.tile([D, 2 * D], E4)
    nc.sync.dma_start(idid[:], idid_d[:])
    bias8 = const.tile([D, 6 * N], E4)
    nc.sync.dma_start(bias8[:], bias8_d[:])
    leb = const.tile([D, 6 * N], F16)
    nc.sync.dma_start(leb[:], leb_d[:])
    # rowsum-replication indicator: row 32 (rs of even head) -> out rows
    # 0..63, row 64 (rs of odd head) -> out rows 64..96; all other rows zero.
    ind97 = const.tile([97, 97], F16)
    nc.vector.memset(ind97[:], 0.0)
    nc.vector.memset(ind97[32:33, 0:64], 1.0)
    nc.vector.memset(ind97[64:65, 64:97], 1.0)

    zrow = const.tile([1, 97], F16)
    nc.vector.memset(zrow[:], 0.0)

    # one-time zero of the ao banks (rows 33..63 are never written by the
    # per-window matmuls but are read by the rsbf cast)
    zA = _ao_tile(ps, "aoA")
    zB = _ao_tile(ps, "aoB")
    nc.tensor.matmul(zA[:, 0:N], lhsT=zrow[:], rhs=leb[0:1, 0:N],
                     start=True, stop=True)
    nc.tensor.matmul(zB[:, 0:N], lhsT=zrow[:], rhs=leb[0:1, 0:N],
                     start=True, stop=True)

    def emit_xt(w):
        xt = sb.tile([D, NP], F16, tag="xt", name="xt")
        nc.sync.dma_start(xt[:], xt_d[w])
        return xt

    def emit_head(w, xt):
        qp = ps.tile([D, N], F32, tag="qk", bufs=1, padded_shape=[D, 512], name="qp")
        nc.tensor.matmul(qp[:], lhsT=wqkv[:, 0:D], rhs=xt[:, 0:N], start=True, stop=True)
        qsb = sb.tile([D, N], F16, tag="qsb", name="qsb")
        nc.scalar.copy(qsb[:], qp[:])                 # ACT cast

        kp = ps.tile([D, NP], F32, tag="qk", bufs=1, padded_shape=[D, 512], name="kp")
        nc.tensor.matmul(kp[:], lhsT=wqkv[:, D:2 * D], rhs=xt[:], start=True, stop=True)
        ksb = sb.tile([D, NP], F16, tag="ksb", name="ksb")
        nc.scalar.copy(ksb[:], kp[:])                 # ACT cast

        vp = ps.tile([D, 3 * D], F32, tag="qk", bufs=1, padded_shape=[D, 512], name="vp")
        v_mms = []
        for c in range(3):
            v_mms.append(nc.tensor.matmul(
                vp[:, c * D:(c + 1) * D],
                lhsT=xt[:, JOFF[c]:JOFF[c] + D],
                rhs=wqkv[:, 2 * D:3 * D],
                start=(c == 0), stop=(c == 2),
            ))
        _chain(v_mms)

        vsb1 = sb.tile([D, 3 * 132], F16, tag="vsb1", name="vsb1")
        vdst = vsb1[:].rearrange("p (c g s) -> p c g s", c=3, g=2)   # s=66
        vsrc = vp[:].rearrange("p (c g s) -> p c g s", c=3, g=2)     # s=64
        nc.vector.tensor_copy(vdst[:, :, :, 0:32], vsrc[:, :, :, 0:32])
        nc.vector.tensor_copy(vdst[:, :, :, 34:66], vsrc[:, :, :, 32:64])
        nc.gpsimd.memset(vdst[:, :, :, 32:34], 1.0)  # ones cols on Pool
        return qsb, ksb, vsb1

    def emit_chunk_sims(qsb, ksb, c):
        """Pair A (h0,h1): DR bias preload + QK -> batched ACT Exp.
        Pair B (h2,h3): QK -> batched DVE Schraudolph w/ f16 leb."""
        attn = sb.tile([D, H * N], F16, tag="attn", bufs=6, name="attn")

        pA = ps.tile([D, 1024], F32, tag="simA", bufs=1,
                     padded_shape=[D, 1024], name="pA")
        mms = []
        for h in (0, 1):
            sl = pA[:, 512 * h:512 * h + N]
            k = 2 * c + h
            mms.append(nc.tensor.matmul(
                sl,
                lhsT=idid[:].rearrange("p (t m) -> p t m", t=2),
                rhs=bias8[:, k * N:(k + 1) * N]
                    .rearrange("p (t n) -> p t n", t=1)
                    .broadcast_to([D, 2, N]),
                start=True, stop=False,
                perf_mode=mybir.MatmulPerfMode.DoubleRow,
                skip_group_check=True,
            ))
            mms.append(nc.tensor.matmul(
                sl,
                lhsT=ksb[DH * h:DH * (h + 1), JOFF[c]:JOFF[c] + D],
                rhs=qsb[DH * h:DH * (h + 1), 0:N],
                tile_position=(DH * h, 0),
                start=False, stop=True,
                skip_group_check=True,
            ))
        _chain(mms)
        # ONE batched exp over both banks -> attn h0|h1
        nc.scalar.activation(
            attn[:, 0:2 * N].rearrange("p (t n) -> p t n", t=2),
            pA[:].rearrange("p (t n) -> p t n", t=2)[:, :, 0:N],
            mybir.ActivationFunctionType.Exp,
        )

        pB = ps.tile([D, 1024], F32, tag="simB", bufs=1,
                     padded_shape=[D, 1024], name="pB")
        mmsB = []
        for h in (2, 3):
            sl = pB[:, 512 * (h - 2):512 * (h - 2) + N]
            mmsB.append(nc.tensor.matmul(
                sl,
                lhsT=ksb[DH * h:DH * (h + 1), JOFF[c]:JOFF[c] + D],
                rhs=qsb[DH * h:DH * (h + 1), 0:N],
                tile_position=(DH * h, 0),
                start=True, stop=True,
            ))
        # ONE batched DVE Schraudolph over both banks -> attn h2|h3
        nc.vector.affine_then_add(
            attn[:, 2 * N:4 * N].bitcast(I16).rearrange("p (t n) -> p t n", t=2),
            pB[:].rearrange("p (t n) -> p t n", t=2)[:, :, 0:N],
            leb[:, 2 * c * N:(2 * c + 2) * N].rearrange("p (t n) -> p t n", t=2),
            SCHR_A, SCHR_B,
        )
        return attn

    def emit_chunk_aos(aoA, aoB, vsb1, attn, c, ao_mms):
        for h in (0, 1, 2, 3):
            bank = aoA if h < 2 else aoB
            off = 64 * (h % 2)
            ao_mms.append(nc.tensor.matmul(
                bank[off:off + 33, :],
                lhsT=vsb1[:, 132 * c + 33 * h:132 * c + 33 * h + 33],
                rhs=attn[:, N * h:N * (h + 1)],
                tile_position=(0, off),
                start=(c == 0), stop=(c == 2),
                skip_group_check=True,
            ))

    def emit_rsbf(aoA, aoB):
        rsbf = sb.tile([97, 2 * N], F16, tag="rsbf", name="rsbf")
        nc.scalar.copy(rsbf[:, 0:N], aoA[:, 0:N])
        nc.vector.tensor_copy(rsbf[:, N:2 * N], aoB[:, 0:N])
        return rsbf

    def emit_rep(rsbf, half, nm):
        rep = ps.tile([97, N], F32, tag="fin", bufs=1, padded_shape=[97, 512],
                      name=nm)
        nc.tensor.matmul(rep[:], lhsT=ind97[:], rhs=rsbf[:, half * N:(half + 1) * N],
                         start=True, stop=True)
        rc = sb.tile([97, N], F32, tag=nm + "rc", name=nm + "rc")
        nc.vector.reciprocal_approx_fast(rc[:], rep[:])
        return rc

    def emit_anrm(rsbf, rc, half, nm):
        anrm = sb.tile([97, N], F16, tag=nm, name=nm)
        nc.gpsimd.tensor_mul(anrm[:], rsbf[:, half * N:(half + 1) * N], rc[:])
        return anrm

    def emit_fin(w, anrmA, anrmB):
        fp = ps.tile([D, 3 * D], F32, tag="fin", bufs=1, padded_shape=[D, 512],
                     name="fp")
        f_mms = []
        for c in range(3):
            jc = min(D, N - JOFF[c])
            f_mms.append(nc.tensor.matmul(
                fp[0:jc, c * D:(c + 1) * D],
                lhsT=anrmA[:, JOFF[c]:JOFF[c] + jc],
                rhs=woutA[:],
                start=True, stop=False,
                skip_group_check=True,
            ))
            f_mms.append(nc.tensor.matmul(
                fp[0:jc, c * D:(c + 1) * D],
                lhsT=anrmB[:, JOFF[c]:JOFF[c] + jc],
                rhs=woutB[:],
                start=False, stop=True,
                skip_group_check=True,
            ))
        _chain(f_mms)

        fsb = sb.tile([D, 3 * D], F32, tag="fsb", name="fsb")
        cp1 = nc.scalar.copy(fsb[:, 0:2 * D], fp[:, 0:2 * D])
        add_dep_helper(cp1.ins, f_mms[-1].ins, sync=True,
                       reason="read after accumulation group closes")
        nc.vector.tensor_copy(fsb[0:87, 2 * D:3 * D], fp[0:87, 2 * D:3 * D])

        dst01 = out_d[w, 0:256, :].rearrange("(c p) d -> p c d", p=D)
        src01 = fsb[:, 0:256].rearrange("p (c d) -> p c d", c=2)
        nc.sync.dma_start(dst01, src01)
        nc.sync.dma_start(out_d[w, 256:343, :], fsb[0:87, 2 * D:3 * D])

    # software-pipelined main loop: window w's normalize/projection tail is
    # emitted interleaved with window w+1's head so in-order engines never
    # stall on it
    prev = None   # (w, rsbf) of the previous window
    xt_cur = emit_xt(0)
    for w in range(NW):
        xt_next = emit_xt(w + 1) if w + 1 < NW else None
        qsb, ksb, vsb1 = emit_head(w, xt_cur)
        xt_cur = xt_next
        if prev is not None:
            rcA = emit_rep(prev[1], 0, "r1")
        aoA = _ao_tile(ps, "aoA")
        aoB = _ao_tile(ps, "aoB")
        ao_mms = []
        attn0 = emit_chunk_sims(qsb, ksb, 0)
        if prev is not None:
            rcB = emit_rep(prev[1], 1, "r2")
            anrmA = emit_anrm(prev[1], rcA, 0, "anrmA")
        attn1 = emit_chunk_sims(qsb, ksb, 1)
        emit_chunk_aos(aoA, aoB, vsb1, attn0, 0, ao_mms)
        if prev is not None:
            anrmB = emit_anrm(prev[1], rcB, 1, "anrmB")
        attn2 = emit_chunk_sims(qsb, ksb, 2)
        emit_chunk_aos(aoA, aoB, vsb1, attn1, 1, ao_mms)
        if prev is not None:
            emit_fin(prev[0], anrmA, anrmB)
        emit_chunk_aos(aoA, aoB, vsb1, attn2, 2, ao_mms)
        _chain(ao_mms)
        rsbf = emit_rsbf(aoA, aoB)
        prev = (w, rsbf)

    # drain the last window's tail
    rcA = emit_rep(prev[1], 0, "r1")
    rcB = emit_rep(prev[1], 1, "r2")
    anrmA = emit_anrm(prev[1], rcA, 0, "anrmA")
    anrmB = emit_anrm(prev[1], rcB, 1, "anrmB")
    emit_fin(prev[0], anrmA, anrmB)


def _prep_inputs(x, w_qkv, w_out, bias_table, rel_idx):
    x = np.asarray(x, dtype=np.float32)
    w_qkv = np.asarray(w_qkv, dtype=np.float32)
    w_out = np.asarray(w_out, dtype=np.float32)
    bias_table = np.asarray(bias_table, dtype=np.float32)
    rel_idx = np.asarray(rel_idx)

    scale = DH ** -0.5
    wq = w_qkv[:, 0:D] * scale
    wqkv_s = np.concatenate([wq, w_qkv[:, D:3 * D]], axis=1)
    wqkv_f16 = wqkv_s.astype(np.float16)

    # wout97: per bank [h_even rows | zero gap | rs row zero | h_odd rows]
    wout97 = np.zeros((194, D), dtype=np.float32)
    wout97[0:32] = w_out[0:32]          # h0 (anrm rows 0..31)
    wout97[65:97] = w_out[32:64]        # h1 (anrm rows 65..96)
    wout97[97:129] = w_out[64:96]       # h2
    wout97[162:194] = w_out[96:128]     # h3
    wout97_f16 = wout97.astype(np.float16)

    xr = x.reshape(8 * 64, N, D)
    xtf = np.zeros((8 * 64, D, NP), dtype=np.float32)
    xtf[:, :, 0:N] = xr.transpose(0, 2, 1)
    xt = xtf.astype(np.float16).reshape(8, NW, D, NP)

    # idid: two identity matrices side by side (DoubleRow k-tiles)
    idid = np.zeros((D, 2 * D), dtype=np.float32)
    idid[:, 0:D] = np.eye(D)
    idid[:, D:2 * D] = np.eye(D)
    idid_e4 = idid.astype(ml_dtypes.float8_e4m3)

    bias = bias_table[rel_idx]                     # (i, j, h)
    biasT = bias.transpose(1, 2, 0)                # (j, h, i)

    # bias8: halved bias tables for pair-A heads (h0,h1), slot k = 2c+h.
    # The DR preload adds the table twice (broadcast k-tiles), restoring the
    # full bias. Rows beyond j=343 (chunk 2) get -150 -> psum -300 -> exp 0.
    b8 = np.full((D, 6 * N), -150.0, dtype=np.float32)
    for c in range(3):
        jn = min(D, N - JOFF[c])
        for h in (0, 1):
            k = 2 * c + h
            b8[0:jn, k * N:(k + 1) * N] = biasT[JOFF[c]:JOFF[c] + jn, h, :] / 2.0
    bias8_e4 = b8.astype(ml_dtypes.float8_e4m3)

    # leb: f16 Schraudolph bias-fold tables for pair-B heads (h2,h3),
    # slot k = 2c+(h-2); masked (padded-j) rows get LEB_MASK.
    lb = np.full((D, 6 * N), LEB_MASK, dtype=np.float32)
    for c in range(3):
        jn = min(D, N - JOFF[c])
        for h in (2, 3):
            k = 2 * c + (h - 2)
            lb[0:jn, k * N:(k + 1) * N] = SCHR_A * biasT[JOFF[c]:JOFF[c] + jn, h, :]
    leb_f16 = lb.astype(np.float16)

    in_maps = []
    for core in range(8):
        in_maps.append({
            "xt": np.ascontiguousarray(xt[core]),
            "wqkv": wqkv_f16,
            "wout": wout97_f16,
            "idid": idid_e4,
            "bias8": bias8_e4,
            "leb": leb_f16,
        })
    return in_maps


def kernel(x, w_qkv, w_out, bias_table, rel_idx):
    if "nc" not in _cache:
        _cache["nc"] = _build_kernel()
    nc = _cache["nc"]
    in_maps = _prep_inputs(x, w_qkv, w_out, bias_table, rel_idx)
    res = bass_utils.run_bass_kernel_spmd(
        nc, in_maps, core_ids=list(range(8)), trace=TRACE, **TRACE_KWARGS
    )
    _cache["last_result"] = res
    outs = [res.results[c]["out"] for c in range(8)]
    full = np.concatenate(outs, axis=0)             # (512, 343, 128)
    return full.reshape(1, 8, 8, 8, 7, 7, 7, D).astype(np.float32)


# revision 15
# speedup vs baseline: 1.1522x; 1.0152x over previous
"""Trainium2 Bass kernel for windowed 3D attention (nn_Attention_12927851561046).

512 windows of 343-token, 4-head, 32-dim-per-head attention over d=128.
Pure data parallel: 64 windows per core across 8 NeuronCores.

v2 design (vs the eb-multiply baseline):
  - f16 data path everywhere bf16 was used (same matmul/DVE cost, more
    mantissa), f32 psum throughout.
  - rel-pos bias is folded into the sim PSUM *before* the QK matmul via a
    cheap fp8-e4m3 DoubleRow matmul (identity lhsT x bias-table rhs at 0.5
    cycles/row, broadcast rhs so the halved table is added twice). The
    softmax then needs NO per-element bias multiply at all.
  - heads are processed in PSUM pairs (one 2-bank tile per pair):
      pair A (h0,h1): bias-preload + QK accumulate, then ONE batched ACT
        Exp over both banks -> attn f16.
      pair B (h2,h3): QK only, then ONE batched DVE affine_then_add
        (Schraudolph: int16 bits of exp(sim+bias), bias table in f16 rides
        the in1 operand) -> attn f16.
  - PV, rowsum-rides-PV, rep/recip/anrm and the final projection keep the
    baseline structure, all in f16 (rc stays f32 for reciprocal_approx_fast).
  - Pool only does the anrm multiplies + ones memsets (GPSIMD cannot touch
    PSUM, so it cannot help with the psum->sbuf conversions).
"""

import sys
from contextlib import ExitStack

import numpy as np

sys.path.insert(0, "/opt/trn_rl_repo")

import ml_dtypes  # noqa: E402

import concourse.bass as bass  # noqa: E402
import concourse.tile as tile  # noqa: E402
from concourse.tile import add_dep_helper  # noqa: E402
from concourse import bacc, mybir  # noqa: E402
from concourse import bass_utils  # noqa: E402

F16 = mybir.dt.float16
F32 = mybir.dt.float32
I16 = mybir.dt.int16
E4 = mybir.dt.float8e4

NW = 64          # windows per core
N = 343          # tokens per window
D = 128
H = 4
DH = 32
NP = 384         # padded tokens (zeros beyond 343)
JOFF = [0, 128, 256]

# f16-domain Schraudolph exp: bits16 = sim*SCHR_A + SCHR_B (+ f16 leb table)
SCHR_A = 1024.0 / float(np.log(2.0))   # 1477.32
SCHR_B = 15360.0 - 59.0                # rms-calibrated
LEB_MASK = -60000.0                    # masked rows -> bits < 0 -> sat -> -0

TRACE = False
TRACE_KWARGS = {}

_cache = {}


def _build_kernel():
    nc = bacc.Bacc(
        "TRN2",
        target_bir_lowering=False,
        debug=False,
        enable_asserts=False,
        num_devices=8,
    )
    xt_d = nc.dram_tensor("xt", (NW, D, NP), F16, kind="ExternalInput").ap()
    wqkv_d = nc.dram_tensor("wqkv", (D, 3 * D), F16, kind="ExternalInput").ap()
    wout_d = nc.dram_tensor("wout", (2 * 97, D), F16, kind="ExternalInput").ap()
    idid_d = nc.dram_tensor("idid", (D, 2 * D), E4, kind="ExternalInput").ap()
    bias8_d = nc.dram_tensor("bias8", (D, 6 * N), E4, kind="ExternalInput").ap()
    leb_d = nc.dram_tensor("leb", (D, 6 * N), F16, kind="ExternalInput").ap()
    out_d = nc.dram_tensor("out", (NW, N, D), F32, kind="ExternalOutput").ap()

    with tile.TileContext(nc) as tc:
        with ExitStack() as ctx:
            _body(ctx, tc, out_d, xt_d, wqkv_d, wout_d, idid_d, bias8_d, leb_d)

    nc.compile()
    return nc


def _chain(insts):
    for a, b in zip(insts[1:], insts[:-1]):
        add_dep_helper(a.ins, b.ins, sync=False, reason="psum accumulation order")


def _ao_tile(ps, tag):
    return ps.tile([97, N], F32, tag=tag, bufs=1, padded_shape=[97, 512],
                   name=tag)


def _body(ctx, tc, out_d, xt_d, wqkv_d, wout_d, idid_d, bias8_d, leb_d):
    nc = tc.nc

    const = ctx.enter_context(tc.tile_pool(name="const", bufs=1))
    sb = ctx.enter_context(tc.tile_pool(name="sb", bufs=4))
    ps = ctx.enter_context(tc.tile_pool(name="ps", bufs=1, space="PSUM"))

    # constants
    wqkv = const.tile([D, 3 * D], F16)
    nc.sync.dma_start(wqkv[:], wqkv_d[:])
    woutA = const.tile([97, D], F16)
    nc.sync.dma_start(woutA[:], wout_d[0:97, :])
    woutB = const.tile([97, D], F16)
    nc.sync.dma_start(woutB[:], wout_d[97:194, :])
    idid = const.tile([D, 2 * D], E4)
    nc.sync.dma_start(idid[:], idid_d[:])
    bias8 = const.tile([D, 6 * N], E4)
    nc.sync.dma_start(bias8[:], bias8_d[:])
    leb = const.tile([D, 6 * N], F16)
    nc.sync.dma_start(leb[:], leb_d[:])
    # rowsum-replication indicator: row 32 (rs of even head) -> out rows
    # 0..63, row 64 (rs of odd head) -> out rows 64..96; all other rows zero.
    ind97 = const.tile([97, 97], F16)
    nc.vector.memset(ind97[:], 0.0)
    nc.vector.memset(ind97[32:33, 0:64], 1.0)
    nc.vector.memset(ind97[64:65, 64:97], 1.0)

    zrow = const.tile([1, 97], F16)
    nc.vector.memset(zrow[:], 0.0)

    # one-time zero of the ao banks (rows 33..63 are never written by the
    # per-window matmuls but are read by the rsbf cast)
    zA = _ao_tile(ps, "aoA")
    zB = _ao_tile(ps, "aoB")
    nc.tensor.matmul(zA[:, 0:N], lhsT=zrow[:], rhs=leb[0:1, 0:N],
                     start=True, stop=True)
    nc.tensor.matmul(zB[:, 0:N], lhsT=zrow[:], rhs=leb[0:1, 0:N],
                     start=True, stop=True)

    def emit_xt(w):
        # Pool-triggered DMA: separate queue from the SP out-DMAs, so the
        # input prefetch is never head-of-line blocked behind them.
        xt = sb.tile([D, NP], F16, tag="xt", name="xt")
        nc.gpsimd.dma_start(xt[:], xt_d[w])
        return xt

    def emit_qk(xt):
        qp = ps.tile([D, N], F32, tag="qk", bufs=1, padded_shape=[D, 512], name="qp")
        nc.tensor.matmul(qp[:], lhsT=wqkv[:, 0:D], rhs=xt[:, 0:N], start=True, stop=True)
        qsb = sb.tile([D, N], F16, tag="qsb", name="qsb")
        nc.scalar.copy(qsb[:], qp[:])                 # ACT cast
        kp = ps.tile([D, NP], F32, tag="qk", bufs=1, padded_shape=[D, 512], name="kp")
        nc.tensor.matmul(kp[:], lhsT=wqkv[:, D:2 * D], rhs=xt[:], start=True, stop=True)
        ksb = sb.tile([D, NP], F16, tag="ksb", name="ksb")
        nc.scalar.copy(ksb[:], kp[:])                 # ACT cast
        return qsb, ksb

    def emit_vp(xt):
        vp = ps.tile([D, 3 * D], F32, tag="qk", bufs=1, padded_shape=[D, 512], name="vp")
        v_mms = []
        for c in range(3):
            v_mms.append(nc.tensor.matmul(
                vp[:, c * D:(c + 1) * D],
                lhsT=xt[:, JOFF[c]:JOFF[c] + D],
                rhs=wqkv[:, 2 * D:3 * D],
                start=(c == 0), stop=(c == 2),
            ))
        _chain(v_mms)

        vsb1 = sb.tile([D, 3 * 132], F16, tag="vsb1", name="vsb1")
        vdst = vsb1[:].rearrange("p (c g s) -> p c g s", c=3, g=2)   # s=66
        vsrc = vp[:].rearrange("p (c g s) -> p c g s", c=3, g=2)     # s=64
        nc.vector.tensor_copy(vdst[:, :, :, 0:32], vsrc[:, :, :, 0:32])
        nc.vector.tensor_copy(vdst[:, :, :, 34:66], vsrc[:, :, :, 32:64])
        nc.gpsimd.memset(vdst[:, :, :, 32:34], 1.0)  # ones cols on Pool
        return vsb1

    def emit_sims_mms(qsb, ksb, c):
        """Pair A (h0,h1): DR bias preload + QK matmuls.
        Pair B (h2,h3): QK matmuls only."""
        attn = sb.tile([D, H * N], F16, tag="attn", bufs=6, name="attn")

        pA = ps.tile([D, 1024], F32, tag="simA", bufs=1,
                     padded_shape=[D, 1024], name="pA")
        mms = []
        for h in (0, 1):
            sl = pA[:, 512 * h:512 * h + N]
            k = 2 * c + h
            mms.append(nc.tensor.matmul(
                sl,
                lhsT=idid[:].rearrange("p (t m) -> p t m", t=2),
                rhs=bias8[:, k * N:(k + 1) * N]
                    .rearrange("p (t n) -> p t n", t=1)
                    .broadcast_to([D, 2, N]),
                start=True, stop=False,
                perf_mode=mybir.MatmulPerfMode.DoubleRow,
                skip_group_check=True,
            ))
            mms.append(nc.tensor.matmul(
                sl,
                lhsT=ksb[DH * h:DH * (h + 1), JOFF[c]:JOFF[c] + D],
                rhs=qsb[DH * h:DH * (h + 1), 0:N],
                tile_position=(DH * h, 0),
                start=False, stop=True,
                skip_group_check=True,
            ))
        _chain(mms)

        pB = ps.tile([D, 1024], F32, tag="simB", bufs=1,
                     padded_shape=[D, 1024], name="pB")
        for h in (2, 3):
            sl = pB[:, 512 * (h - 2):512 * (h - 2) + N]
            nc.tensor.matmul(
                sl,
                lhsT=ksb[DH * h:DH * (h + 1), JOFF[c]:JOFF[c] + D],
                rhs=qsb[DH * h:DH * (h + 1), 0:N],
                tile_position=(DH * h, 0),
                start=True, stop=True,
            )
        return pA, pB, attn

    def emit_expA(pA, attn):
        # ONE batched exp over both banks -> attn h0|h1
        nc.scalar.activation(
            attn[:, 0:2 * N].rearrange("p (t n) -> p t n", t=2),
            pA[:].rearrange("p (t n) -> p t n", t=2)[:, :, 0:N],
            mybir.ActivationFunctionType.Exp,
        )

    def emit_affB(pB, attn, c):
        # ONE batched DVE Schraudolph over both banks -> attn h2|h3
        nc.vector.affine_then_add(
            attn[:, 2 * N:4 * N].bitcast(I16).rearrange("p (t n) -> p t n", t=2),
            pB[:].rearrange("p (t n) -> p t n", t=2)[:, :, 0:N],
            leb[:, 2 * c * N:(2 * c + 2) * N].rearrange("p (t n) -> p t n", t=2),
            SCHR_A, SCHR_B,
        )

    def emit_chunk_aos(aoA, aoB, vsb1, attn, c, ao_mms):
        for h in (0, 1, 2, 3):
            bank = aoA if h < 2 else aoB
            off = 64 * (h % 2)
            ao_mms.append(nc.tensor.matmul(
                bank[off:off + 33, :],
                lhsT=vsb1[:, 132 * c + 33 * h:132 * c + 33 * h + 33],
                rhs=attn[:, N * h:N * (h + 1)],
                tile_position=(0, off),
                start=(c == 0), stop=(c == 2),
                skip_group_check=True,
            ))

    def emit_rsbf(aoA, aoB):
        rsbf = sb.tile([97, 2 * N], F16, tag="rsbf", name="rsbf")
        nc.scalar.copy(rsbf[:, 0:N], aoA[:, 0:N])
        nc.scalar.copy(rsbf[:, N:2 * N], aoB[:, 0:N])
        return rsbf

    def emit_rep(rsbf, half, nm):
        rep = ps.tile([97, N], F32, tag="aoA" if half == 0 else "aoB", bufs=1,
                      padded_shape=[97, 512], name=nm)
        nc.tensor.matmul(rep[:], lhsT=ind97[:], rhs=rsbf[:, half * N:(half + 1) * N],
                         start=True, stop=True)
        rc = sb.tile([97, N], F32, tag=nm + "rc", name=nm + "rc")
        nc.vector.reciprocal_approx_fast(rc[:], rep[:])
        return rc

    def emit_anrm(rsbf, rc, half, nm):
        anrm = sb.tile([97, N], F16, tag=nm, name=nm)
        nc.gpsimd.tensor_mul(anrm[:], rsbf[:, half * N:(half + 1) * N], rc[:])
        return anrm

    def emit_fin(w, anrmA, anrmB):
        fp = ps.tile([D, 3 * D], F32, tag="fin", bufs=1, padded_shape=[D, 512],
                     name="fp")
        f_mms = []
        for c in range(3):
            jc = min(D, N - JOFF[c])
            f_mms.append(nc.tensor.matmul(
                fp[0:jc, c * D:(c + 1) * D],
                lhsT=anrmA[:, JOFF[c]:JOFF[c] + jc],
                rhs=woutA[:],
                start=True, stop=False,
                skip_group_check=True,
            ))
            f_mms.append(nc.tensor.matmul(
                fp[0:jc, c * D:(c + 1) * D],
                lhsT=anrmB[:, JOFF[c]:JOFF[c] + jc],
                rhs=woutB[:],
                start=False, stop=True,
                skip_group_check=True,
            ))
        _chain(f_mms)

        fsb = sb.tile([D, 3 * D], F32, tag="fsb", name="fsb")
        cp1 = nc.scalar.copy(fsb[:, 0:2 * D], fp[:, 0:2 * D])
        add_dep_helper(cp1.ins, f_mms[-1].ins, sync=True,
                       reason="read after accumulation group closes")
        nc.vector.tensor_copy(fsb[0:87, 2 * D:3 * D], fp[0:87, 2 * D:3 * D])

        dst01 = out_d[w, 0:256, :].rearrange("(c p) d -> p c d", p=D)
        src01 = fsb[:, 0:256].rearrange("p (c d) -> p c d", c=2)
        nc.sync.dma_start(dst01, src01)
        nc.sync.dma_start(out_d[w, 256:343, :], fsb[0:87, 2 * D:3 * D])

    # two-deep software pipeline: iteration w emits window w's sims/PV,
    # window w-1's normalize/projection tail, and window w+1's q/k/v head.
    # The head casts are interleaved into the chunk phases so the ACT/DVE
    # queues have them ready before the next iteration's sim matmuls.
    prev = None   # (w, rsbf) of the previous window
    xt_cur = emit_xt(0)
    qsb, ksb = emit_qk(xt_cur)
    vsb1 = emit_vp(xt_cur)
    xt_next = emit_xt(1)
    for w in range(NW):
        if prev is not None:
            rcA = emit_rep(prev[1], 0, "r1")
        aoA = _ao_tile(ps, "aoA")
        aoB = _ao_tile(ps, "aoB")
        ao_mms = []
        pA0, pB0, attn0 = emit_sims_mms(qsb, ksb, 0)
        emit_expA(pA0, attn0)
        emit_affB(pB0, attn0, 0)
        if prev is not None:
            rcB = emit_rep(prev[1], 1, "r2")
            anrmA = emit_anrm(prev[1], rcA, 0, "anrmA")
        pA1, pB1, attn1 = emit_sims_mms(qsb, ksb, 1)
        emit_expA(pA1, attn1)
        emit_affB(pB1, attn1, 1)
        emit_chunk_aos(aoA, aoB, vsb1, attn0, 0, ao_mms)
        if prev is not None:
            anrmB = emit_anrm(prev[1], rcB, 1, "anrmB")
        last = w + 1 >= NW
        pA2, pB2, attn2 = emit_sims_mms(qsb, ksb, 2)
        if not last:
            qsb_n, ksb_n = emit_qk(xt_next)   # ACT casts land before exp2
        emit_expA(pA2, attn2)
        emit_affB(pB2, attn2, 2)
        emit_chunk_aos(aoA, aoB, vsb1, attn1, 1, ao_mms)
        if prev is not None:
            emit_fin(prev[0], anrmA, anrmB)
        emit_chunk_aos(aoA, aoB, vsb1, attn2, 2, ao_mms)
        _chain(ao_mms)
        if not last:
            vsb1_n = emit_vp(xt_next)
        rsbf = emit_rsbf(aoA, aoB)
        prev = (w, rsbf)
        if not last:
            qsb, ksb, vsb1 = qsb_n, ksb_n, vsb1_n
            xt_next = emit_xt(w + 2) if w + 2 < NW else None

    # drain the last window's tail
    rcA = emit_rep(prev[1], 0, "r1")
    rcB = emit_rep(prev[1], 1, "r2")
    anrmA = emit_anrm(prev[1], rcA, 0, "anrmA")
    anrmB = emit_anrm(prev[1], rcB, 1, "anrmB")
    emit_fin(prev[0], anrmA, anrmB)


def _prep_inputs(x, w_qkv, w_out, bias_table, rel_idx):
    x = np.asarray(x, dtype=np.float32)
    w_qkv = np.asarray(w_qkv, dtype=np.float32)
    w_out = np.asarray(w_out, dtype=np.float32)
    bias_table = np.asarray(bias_table, dtype=np.float32)
    rel_idx = np.asarray(rel_idx)

    scale = DH ** -0.5
    wq = w_qkv[:, 0:D] * scale
    wqkv_s = np.concatenate([wq, w_qkv[:, D:3 * D]], axis=1)
    wqkv_f16 = wqkv_s.astype(np.float16)

    # wout97: per bank [h_even rows | zero gap | rs row zero | h_odd rows]
    wout97 = np.zeros((194, D), dtype=np.float32)
    wout97[0:32] = w_out[0:32]          # h0 (anrm rows 0..31)
    wout97[65:97] = w_out[32:64]        # h1 (anrm rows 65..96)
    wout97[97:129] = w_out[64:96]       # h2
    wout97[162:194] = w_out[96:128]     # h3
    wout97_f16 = wout97.astype(np.float16)

    xr = x.reshape(8 * 64, N, D)
    xtf = np.zeros((8 * 64, D, NP), dtype=np.float32)
    xtf[:, :, 0:N] = xr.transpose(0, 2, 1)
    xt = xtf.astype(np.float16).reshape(8, NW, D, NP)

    # idid: two identity matrices side by side (DoubleRow k-tiles)
    idid = np.zeros((D, 2 * D), dtype=np.float32)
    idid[:, 0:D] = np.eye(D)
    idid[:, D:2 * D] = np.eye(D)
    idid_e4 = idid.astype(ml_dtypes.float8_e4m3)

    bias = bias_table[rel_idx]                     # (i, j, h)
    biasT = bias.transpose(1, 2, 0)                # (j, h, i)

    # bias8: halved bias tables for pair-A heads (h0,h1), slot k = 2c+h.
    # The DR preload adds the table twice (broadcast k-tiles), restoring the
    # full bias. Rows beyond j=343 (chunk 2) get -150 -> psum -300 -> exp 0.
    b8 = np.full((D, 6 * N), -150.0, dtype=np.float32)
    for c in range(3):
        jn = min(D, N - JOFF[c])
        for h in (0, 1):
            k = 2 * c + h
            b8[0:jn, k * N:(k + 1) * N] = biasT[JOFF[c]:JOFF[c] + jn, h, :] / 2.0
    bias8_e4 = b8.astype(ml_dtypes.float8_e4m3)

    # leb: f16 Schraudolph bias-fold tables for pair-B heads (h2,h3),
    # slot k = 2c+(h-2); masked (padded-j) rows get LEB_MASK.
    lb = np.full((D, 6 * N), LEB_MASK, dtype=np.float32)
    for c in range(3):
        jn = min(D, N - JOFF[c])
        for h in (2, 3):
            k = 2 * c + (h - 2)
            lb[0:jn, k * N:(k + 1) * N] = SCHR_A * biasT[JOFF[c]:JOFF[c] + jn, h, :]
    leb_f16 = lb.astype(np.float16)

    in_maps = []
    for core in range(8):
        in_maps.append({
            "xt": np.ascontiguousarray(xt[core]),
            "wqkv": wqkv_f16,
            "wout": wout97_f16,
            "idid": idid_e4,
            "bias8": bias8_e4,
            "leb": leb_f16,
        })
    return in_maps


def kernel(x, w_qkv, w_out, bias_table, rel_idx):
    if "nc" not in _cache:
        _cache["nc"] = _build_kernel()
    nc = _cache["nc"]
    in_maps = _prep_inputs(x, w_qkv, w_out, bias_table, rel_idx)
    res = bass_utils.run_bass_kernel_spmd(
        nc, in_maps, core_ids=list(range(8)), trace=TRACE, **TRACE_KWARGS
    )
    _cache["last_result"] = res
    outs = [res.results[c]["out"] for c in range(8)]
    full = np.concatenate(outs, axis=0)             # (512, 343, 128)
    return full.reshape(1, 8, 8, 8, 7, 7, 7, D).astype(np.float32)


# revision 26
# speedup vs baseline: 1.1525x; 1.0002x over previous
"""Trainium2 Bass kernel for windowed 3D attention (nn_Attention_12927851561046).

512 windows of 343-token, 4-head, 32-dim-per-head attention over d=128.
Pure data parallel: 64 windows per core across 8 NeuronCores.

v2 design (vs the eb-multiply baseline):
  - f16 data path everywhere bf16 was used (same matmul/DVE cost, more
    mantissa), f32 psum throughout.
  - rel-pos bias is folded into the sim PSUM *before* the QK matmul via a
    cheap fp8-e4m3 DoubleRow matmul (identity lhsT x bias-table rhs at 0.5
    cycles/row, broadcast rhs so the halved table is added twice). The
    softmax then needs NO per-element bias multiply at all.
  - heads are processed in PSUM pairs (one 2-bank tile per pair):
      pair A (h0,h1): bias-preload + QK accumulate, then ONE batched ACT
        Exp over both banks -> attn f16.
      pair B (h2,h3): QK only, then ONE batched DVE affine_then_add
        (Schraudolph: int16 bits of exp(sim+bias), bias table in f16 rides
        the in1 operand) -> attn f16.
  - PV, rowsum-rides-PV, rep/recip/anrm and the final projection keep the
    baseline structure, all in f16 (rc stays f32 for reciprocal_approx_fast).
  - Pool only does the anrm multiplies + ones memsets (GPSIMD cannot touch
    PSUM, so it cannot help with the psum->sbuf conversions).
"""

import sys
from contextlib import ExitStack

import numpy as np

sys.path.insert(0, "/opt/trn_rl_repo")

import ml_dtypes  # noqa: E402

import concourse.bass as bass  # noqa: E402
import concourse.tile as tile  # noqa: E402
from concourse.tile import add_dep_helper  # noqa: E402
from concourse import bacc, mybir  # noqa: E402
from concourse import bass_utils  # noqa: E402

F16 = mybir.dt.float16
F32 = mybir.dt.float32
I16 = mybir.dt.int16
E4 = mybir.dt.float8e4

NW = 64          # windows per core
N = 343          # tokens per window
D = 128
H = 4
DH = 32
NP = 384         # padded tokens (zeros beyond 343)
JOFF = [0, 128, 256]

# f16-domain Schraudolph exp: bits16 = sim*SCHR_A + SCHR_B (+ f16 leb table)
SCHR_A = 1024.0 / float(np.log(2.0))   # 1477.32
SCHR_B = 15360.0 - 59.0                # rms-calibrated
LEB_MASK = -60000.0                    # masked rows -> bits < 0 -> sat -> -0

TRACE = False
TRACE_KWARGS = {}

_cache = {}


def _build_kernel():
    nc = bacc.Bacc(
        "TRN2",
        target_bir_lowering=False,
        debug=False,
        enable_asserts=False,
        num_devices=8,
    )
    xt_d = nc.dram_tensor("xt", (NW, D, NP), F16, kind="ExternalInput").ap()
    wqkv_d = nc.dram_tensor("wqkv", (D, 3 * D), F16, kind="ExternalInput").ap()
    wout_d = nc.dram_tensor("wout", (2 * 97, D), F16, kind="ExternalInput").ap()
    idid_d = nc.dram_tensor("idid", (D, 2 * D), E4, kind="ExternalInput").ap()
    bias8_d = nc.dram_tensor("bias8", (D, 6 * N), E4, kind="ExternalInput").ap()
    leb_d = nc.dram_tensor("leb", (D, 6 * N), F16, kind="ExternalInput").ap()
    out_d = nc.dram_tensor("out", (NW, N, D), F32, kind="ExternalOutput").ap()

    with tile.TileContext(nc) as tc:
        with ExitStack() as ctx:
            _body(ctx, tc, out_d, xt_d, wqkv_d, wout_d, idid_d, bias8_d, leb_d)

    nc.compile()
    return nc


def _chain(insts):
    for a, b in zip(insts[1:], insts[:-1]):
        add_dep_helper(a.ins, b.ins, sync=False, reason="psum accumulation order")


def _ao_tile(ps, tag):
    return ps.tile([97, N], F32, tag=tag, bufs=1, padded_shape=[97, 512],
                   name=tag)


def _body(ctx, tc, out_d, xt_d, wqkv_d, wout_d, idid_d, bias8_d, leb_d):
    nc = tc.nc

    const = ctx.enter_context(tc.tile_pool(name="const", bufs=1))
    sb = ctx.enter_context(tc.tile_pool(name="sb", bufs=4))
    ps = ctx.enter_context(tc.tile_pool(name="ps", bufs=1, space="PSUM"))

    # constants
    wqkv = const.tile([D, 3 * D], F16)
    nc.sync.dma_start(wqkv[:], wqkv_d[:])
    woutA = const.tile([97, D], F16)
    nc.sync.dma_start(woutA[:], wout_d[0:97, :])
    woutB = const.tile([97, D], F16)
    nc.sync.dma_start(woutB[:], wout_d[97:194, :])
    idid = const.tile([D, 2 * D], E4)
    nc.sync.dma_start(idid[:], idid_d[:])
    bias8 = const.tile([D, 6 * N], E4)
    nc.sync.dma_start(bias8[:], bias8_d[:])
    leb = const.tile([D, 6 * N], F16)
    nc.sync.dma_start(leb[:], leb_d[:])
    # rowsum-replication indicator: row 32 (rs of even head) -> out rows
    # 0..63, row 64 (rs of odd head) -> out rows 64..96; all other rows zero.
    ind97 = const.tile([97, 97], F16)
    nc.vector.memset(ind97[:], 0.0)
    nc.vector.memset(ind97[32:33, 0:64], 1.0)
    nc.vector.memset(ind97[64:65, 64:97], 1.0)

    zrow = const.tile([1, 97], F16)
    nc.vector.memset(zrow[:], 0.0)

    # one-time zero of the ao banks (rows 33..63 are never written by the
    # per-window matmuls but are read by the rsbf cast)
    zA = _ao_tile(ps, "aoA")
    zB = _ao_tile(ps, "aoB")
    nc.tensor.matmul(zA[:, 0:N], lhsT=zrow[:], rhs=leb[0:1, 0:N],
                     start=True, stop=True)
    nc.tensor.matmul(zB[:, 0:N], lhsT=zrow[:], rhs=leb[0:1, 0:N],
                     start=True, stop=True)

    def emit_xt(w):
        # Pool-triggered DMA: separate queue from the SP out-DMAs, so the
        # input prefetch is never head-of-line blocked behind them.
        xt = sb.tile([D, NP], F16, tag="xt", name="xt")
        nc.gpsimd.dma_start(xt[:], xt_d[w])
        return xt

    def emit_qk(xt):
        qp = ps.tile([D, N], F32, tag="qk", bufs=1, padded_shape=[D, 512], name="qp")
        nc.tensor.matmul(qp[:], lhsT=wqkv[:, 0:D], rhs=xt[:, 0:N], start=True, stop=True)
        qsb = sb.tile([D, N], F16, tag="qsb", name="qsb")
        nc.scalar.copy(qsb[:], qp[:])                 # ACT cast
        kp = ps.tile([D, NP], F32, tag="qk", bufs=1, padded_shape=[D, 512], name="kp")
        nc.tensor.matmul(kp[:], lhsT=wqkv[:, D:2 * D], rhs=xt[:], start=True, stop=True)
        ksb = sb.tile([D, NP], F16, tag="ksb", name="ksb")
        nc.scalar.copy(ksb[:], kp[:])                 # ACT cast
        return qsb, ksb

    def emit_vp(xt):
        vp = ps.tile([D, 3 * D], F32, tag="qk", bufs=1, padded_shape=[D, 512], name="vp")
        v_mms = []
        for c in range(3):
            v_mms.append(nc.tensor.matmul(
                vp[:, c * D:(c + 1) * D],
                lhsT=xt[:, JOFF[c]:JOFF[c] + D],
                rhs=wqkv[:, 2 * D:3 * D],
                start=(c == 0), stop=(c == 2),
            ))
        _chain(v_mms)

        vsb1 = sb.tile([D, 3 * 132], F16, tag="vsb1", name="vsb1")
        vdst = vsb1[:].rearrange("p (c g s) -> p c g s", c=3, g=2)   # s=66
        vsrc = vp[:].rearrange("p (c g s) -> p c g s", c=3, g=2)     # s=64
        nc.vector.tensor_copy(vdst[:, :, :, 0:32], vsrc[:, :, :, 0:32])
        nc.vector.tensor_copy(vdst[:, :, :, 34:66], vsrc[:, :, :, 32:64])
        nc.gpsimd.memset(vdst[:, :, :, 32:34], 1.0)  # ones cols on Pool
        return vsb1

    def act_heads(c):
        return (0, 1)

    def emit_sims_mms(qsb, ksb, c):
        """ACT pair: DR bias preload + QK matmuls -> batched Exp.
        DVE pair: QK matmuls only -> batched Schraudolph."""
        attn = sb.tile([D, H * N], F16, tag="attn", bufs=8, name="attn")
        hA = act_heads(c)

        pA = ps.tile([D, 1024], F32, tag="simA", bufs=1,
                     padded_shape=[D, 1024], name="pA")
        mms = []
        for i, h in enumerate(hA):
            sl = pA[:, 512 * i:512 * i + N]
            k = 2 * c + i
            mms.append(nc.tensor.matmul(
                sl,
                lhsT=idid[:].rearrange("p (t m) -> p t m", t=2),
                rhs=bias8[:, k * N:(k + 1) * N]
                    .rearrange("p (t n) -> p t n", t=1)
                    .broadcast_to([D, 2, N]),
                start=True, stop=False,
                perf_mode=mybir.MatmulPerfMode.DoubleRow,
                skip_group_check=True,
            ))
            mms.append(nc.tensor.matmul(
                sl,
                lhsT=ksb[DH * h:DH * (h + 1), JOFF[c]:JOFF[c] + D],
                rhs=qsb[DH * h:DH * (h + 1), 0:N],
                tile_position=(DH * h, 0),
                start=False, stop=True,
                skip_group_check=True,
            ))
        _chain(mms)

        pB = ps.tile([D, 1024], F32, tag="simB", bufs=1,
                     padded_shape=[D, 1024], name="pB")
        hB = (0, 1) if hA == (2, 3) else (2, 3)
        for i, h in enumerate(hB):
            sl = pB[:, 512 * i:512 * i + N]
            nc.tensor.matmul(
                sl,
                lhsT=ksb[DH * h:DH * (h + 1), JOFF[c]:JOFF[c] + D],
                rhs=qsb[DH * h:DH * (h + 1), 0:N],
                tile_position=(DH * h, 0),
                start=True, stop=True,
            )
        return pA, pB, attn

    def emit_expA(pA, attn, c):
        hA = act_heads(c)
        nc.scalar.activation(
            attn[:, hA[0] * N:(hA[0] + 2) * N].rearrange("p (t n) -> p t n", t=2),
            pA[:].rearrange("p (t n) -> p t n", t=2)[:, :, 0:N],
            mybir.ActivationFunctionType.Exp,
        )

    def emit_affB(pB, attn, c):
        hB = (0, 1) if act_heads(c) == (2, 3) else (2, 3)
        nc.vector.affine_then_add(
            attn[:, hB[0] * N:(hB[0] + 2) * N].bitcast(I16)
                .rearrange("p (t n) -> p t n", t=2),
            pB[:].rearrange("p (t n) -> p t n", t=2)[:, :, 0:N],
            leb[:, 2 * c * N:(2 * c + 2) * N].rearrange("p (t n) -> p t n", t=2),
            SCHR_A, SCHR_B,
        )

    def emit_chunk_aos(aoA, aoB, vsb1, attn, c, ao_mms):
        for h in (0, 1, 2, 3):
            bank = aoA if h < 2 else aoB
            off = 64 * (h % 2)
            ao_mms.append(nc.tensor.matmul(
                bank[off:off + 33, :],
                lhsT=vsb1[:, 132 * c + 33 * h:132 * c + 33 * h + 33],
                rhs=attn[:, N * h:N * (h + 1)],
                tile_position=(0, off),
                start=(c == 0), stop=(c == 2),
                skip_group_check=True,
            ))

    def emit_rsbf(aoA, aoB):
        rsbf = sb.tile([97, 2 * N], F16, tag="rsbf", name="rsbf")
        nc.scalar.copy(rsbf[:, 0:N], aoA[:, 0:N])
        nc.scalar.copy(rsbf[:, N:2 * N], aoB[:, 0:N])
        return rsbf

    def emit_rep(rsbf, half, nm):
        rep = ps.tile([97, N], F32, tag="aoA" if half == 0 else "aoB", bufs=1,
                      padded_shape=[97, 512], name=nm)
        nc.tensor.matmul(rep[:], lhsT=ind97[:], rhs=rsbf[:, half * N:(half + 1) * N],
                         start=True, stop=True)
        rc = sb.tile([97, N], F32, tag=nm + "rc", name=nm + "rc")
        nc.vector.reciprocal_approx_fast(rc[:], rep[:])
        return rc

    def emit_anrm(rsbf, rc, half, nm):
        anrm = sb.tile([97, N], F16, tag=nm, name=nm)
        nc.gpsimd.tensor_mul(anrm[:], rsbf[:, half * N:(half + 1) * N], rc[:])
        return anrm

    def emit_fin(w, anrmA, anrmB):
        fp = ps.tile([D, 3 * D], F32, tag="fin", bufs=1, padded_shape=[D, 512],
                     name="fp")
        f_mms = []
        for c in range(3):
            jc = min(D, N - JOFF[c])
            f_mms.append(nc.tensor.matmul(
                fp[0:jc, c * D:(c + 1) * D],
                lhsT=anrmA[:, JOFF[c]:JOFF[c] + jc],
                rhs=woutA[:],
                start=True, stop=False,
                skip_group_check=True,
            ))
            f_mms.append(nc.tensor.matmul(
                fp[0:jc, c * D:(c + 1) * D],
                lhsT=anrmB[:, JOFF[c]:JOFF[c] + jc],
                rhs=woutB[:],
                start=False, stop=True,
                skip_group_check=True,
            ))
        _chain(f_mms)

        fsb = sb.tile([D, 3 * D], F32, tag="fsb", name="fsb")
        cp1 = nc.scalar.copy(fsb[:, 0:2 * D], fp[:, 0:2 * D])
        add_dep_helper(cp1.ins, f_mms[-1].ins, sync=True,
                       reason="read after accumulation group closes")
        nc.vector.tensor_copy(fsb[0:87, 2 * D:3 * D], fp[0:87, 2 * D:3 * D])

        dst01 = out_d[w, 0:256, :].rearrange("(c p) d -> p c d", p=D)
        src01 = fsb[:, 0:256].rearrange("p (c d) -> p c d", c=2)
        nc.sync.dma_start(dst01, src01)
        nc.sync.dma_start(out_d[w, 256:343, :], fsb[0:87, 2 * D:3 * D])

    # two-deep software pipeline: iteration w emits window w's sims/PV,
    # window w-1's normalize/projection tail, and window w+1's q/k/v head.
    # The head casts are interleaved into the chunk phases so the ACT/DVE
    # queues have them ready before the next iteration's sim matmuls.
    prev = None   # (w, rsbf) of the previous window
    xt_cur = emit_xt(0)
    qsb, ksb = emit_qk(xt_cur)
    vsb1 = emit_vp(xt_cur)
    xt_next = emit_xt(1)
    for w in range(NW):
        if prev is not None:
            rcA = emit_rep(prev[1], 0, "r1")
        aoA = _ao_tile(ps, "aoA")
        aoB = _ao_tile(ps, "aoB")
        ao_mms = []
        pA0, pB0, attn0 = emit_sims_mms(qsb, ksb, 0)
        emit_expA(pA0, attn0, 0)
        emit_affB(pB0, attn0, 0)
        if prev is not None:
            rcB = emit_rep(prev[1], 1, "r2")
            anrmA = emit_anrm(prev[1], rcA, 0, "anrmA")
        pA1, pB1, attn1 = emit_sims_mms(qsb, ksb, 1)
        emit_expA(pA1, attn1, 1)
        emit_affB(pB1, attn1, 1)
        emit_chunk_aos(aoA, aoB, vsb1, attn0, 0, ao_mms)
        if prev is not None:
            anrmB = emit_anrm(prev[1], rcB, 1, "anrmB")
        last = w + 1 >= NW
        pA2, pB2, attn2 = emit_sims_mms(qsb, ksb, 2)
        if not last:
            qsb_n, ksb_n = emit_qk(xt_next)   # ACT casts land before exp2
        emit_expA(pA2, attn2, 2)
        emit_affB(pB2, attn2, 2)
        emit_chunk_aos(aoA, aoB, vsb1, attn1, 1, ao_mms)
        if prev is not None:
            emit_fin(prev[0], anrmA, anrmB)
        emit_chunk_aos(aoA, aoB, vsb1, attn2, 2, ao_mms)
        _chain(ao_mms)
        if not last:
            vsb1_n = emit_vp(xt_next)
        rsbf = emit_rsbf(aoA, aoB)
        prev = (w, rsbf)
        if not last:
            qsb, ksb, vsb1 = qsb_n, ksb_n, vsb1_n
            xt_next = emit_xt(w + 2) if w + 2 < NW else None

    # drain the last window's tail
    rcA = emit_rep(prev[1], 0, "r1")
    rcB = emit_rep(prev[1], 1, "r2")
    anrmA = emit_anrm(prev[1], rcA, 0, "anrmA")
    anrmB = emit_anrm(prev[1], rcB, 1, "anrmB")
    emit_fin(prev[0], anrmA, anrmB)


def _prep_inputs(x, w_qkv, w_out, bias_table, rel_idx):
    x = np.asarray(x, dtype=np.float32)
    w_qkv = np.asarray(w_qkv, dtype=np.float32)
    w_out = np.asarray(w_out, dtype=np.float32)
    bias_table = np.asarray(bias_table, dtype=np.float32)
    rel_idx = np.asarray(rel_idx)

    scale = DH ** -0.5
    wq = w_qkv[:, 0:D] * scale
    wqkv_s = np.concatenate([wq, w_qkv[:, D:3 * D]], axis=1)
    wqkv_f16 = wqkv_s.astype(np.float16)

    # wout97: per bank [h_even rows | zero gap | rs row zero | h_odd rows]
    wout97 = np.zeros((194, D), dtype=np.float32)
    wout97[0:32] = w_out[0:32]          # h0 (anrm rows 0..31)
    wout97[65:97] = w_out[32:64]        # h1 (anrm rows 65..96)
    wout97[97:129] = w_out[64:96]       # h2
    wout97[162:194] = w_out[96:128]     # h3
    wout97_f16 = wout97.astype(np.float16)

    xr = x.reshape(8 * 64, N, D)
    xtf = np.zeros((8 * 64, D, NP), dtype=np.float32)
    xtf[:, :, 0:N] = xr.transpose(0, 2, 1)
    xt = xtf.astype(np.float16).reshape(8, NW, D, NP)

    # idid: two identity matrices side by side (DoubleRow k-tiles)
    idid = np.zeros((D, 2 * D), dtype=np.float32)
    idid[:, 0:D] = np.eye(D)
    idid[:, D:2 * D] = np.eye(D)
    idid_e4 = idid.astype(ml_dtypes.float8_e4m3)

    bias = bias_table[rel_idx]                     # (i, j, h)
    biasT = bias.transpose(1, 2, 0)                # (j, h, i)

    # bias8: halved bias tables for pair-A heads (h0,h1), slot k = 2c+h.
    # The DR preload adds the table twice (broadcast k-tiles), restoring the
    # full bias. Rows beyond j=343 (chunk 2) get -150 -> psum -300 -> exp 0.
    b8 = np.full((D, 6 * N), -150.0, dtype=np.float32)
    for c in range(3):
        jn = min(D, N - JOFF[c])
        hA = (0, 1)                            # ACT-pair heads per chunk
        for i, h in enumerate(hA):
            k = 2 * c + i
            b8[0:jn, k * N:(k + 1) * N] = biasT[JOFF[c]:JOFF[c] + jn, h, :] / 2.0
    bias8_e4 = b8.astype(ml_dtypes.float8_e4m3)

    # leb: f16 Schraudolph bias-fold tables for pair-B heads (h2,h3),
    # slot k = 2c+(h-2); masked (padded-j) rows get LEB_MASK.
    lb = np.full((D, 6 * N), LEB_MASK, dtype=np.float32)
    for c in range(3):
        jn = min(D, N - JOFF[c])
        hB = (2, 3)                            # DVE-pair heads per chunk
        for i, h in enumerate(hB):
            k = 2 * c + i
            lb[0:jn, k * N:(k + 1) * N] = SCHR_A * biasT[JOFF[c]:JOFF[c] + jn, h, :]
    leb_f16 = lb.astype(np.float16)

    in_maps = []
    for core in range(8):
        in_maps.append({
            "xt": np.ascontiguousarray(xt[core]),
            "wqkv": wqkv_f16,
            "wout": wout97_f16,
            "idid": idid_e4,
            "bias8": bias8_e4,
            "leb": leb_f16,
        })
    return in_maps


def kernel(x, w_qkv, w_out, bias_table, rel_idx):
    if "nc" not in _cache:
        _cache["nc"] = _build_kernel()
    nc = _cache["nc"]
    in_maps = _prep_inputs(x, w_qkv, w_out, bias_table, rel_idx)
    res = bass_utils.run_bass_kernel_spmd(
        nc, in_maps, core_ids=list(range(8)), trace=TRACE, **TRACE_KWARGS
    )
    _cache["last_result"] = res
    outs = [res.results[c]["out"] for c in range(8)]
    full = np.concatenate(outs, axis=0)             # (512, 343, 128)
    return full.reshape(1, 8, 8, 8, 7, 7, 7, D).astype(np.float32)
